# revision 14
# baseline (speedup 1.0000x reference)
"""GQA (H=32, KV=8, D=128, T=2048, hid=4096) fp32 causal attention + RoPE,
tensor-parallel over heads across 8 NeuronCores.

v3: bf16 matmul datapath (fp32 accumulate) with a fully software-pipelined
schedule tuned from perfetto traces:
  - Phase 1 (QKV+RoPE): per-t-chunk epilogue reordered so the 6 PSUM
    accumulator banks free within ~1us of the last matmul (cos-mult on DVE +
    copy on ACT lead), V-transposes moved inside the chunk loop to fill the
    PE boundary bubble, RoPE shuffle DMAs moved to the gpsimd queue so the
    sync queue keeps streaming x tiles.
  - Phase 2 (attention): S matmuls + exp (ACT) + causal tri-mask (DVE, only
    the 128 leading cols of diagonal tiles) + PV, with the softmax
    denominator accumulated on DVE (f32r) and reduced across partitions by 4
    col-tiled f32r matmuls per chunk. Diagonal tiles column-restricted
    (N = 512-128m).
  - Per q-chunk AllGather [512,512]bf16 -> [4096,512], first-use latency
    absorbed by a dummy collective during phase 1.
  - Phase 3 (o_proj): whole-chunk activation strip DMA'd from shared HBM on
    the gpsimd queue; matmul tiles interleaved INTO phase 2's kt loop two
    chunks later (gather latency pipeline depth 2) to keep the PE dense.
Host concatenates the 8 column slices.
"""

import math
import numpy as np
import ml_dtypes

import concourse.bass as bass
import concourse.mybir as mybir
import concourse.tile as tile
from concourse import bacc
from concourse.bass_utils import run_bass_kernel_spmd

T = 2048
HID = 4096
H = 32
KV = 8
D = 128
NC = 8
HQ = H // NC          # 4 query heads per core
DQ = HQ * D           # 512
KT = HID // 128       # 32 contraction tiles
TC = T // 512         # 4 t-chunks
ROPE_BASE = 10000.0

BF16 = mybir.dt.bfloat16
F32 = mybir.dt.float32
F32R = mybir.dt.float32r
NPBF16 = ml_dtypes.bfloat16

_BUILD_CACHE = {}
RUN_KWARGS = {}  # test harness hook (e.g. {"trace": True})


def _build_nc():
    nc = bacc.Bacc(None, target_bir_lowering=False, num_devices=NC)

    xT = nc.declare_dram_parameter("xT", [HID, T], BF16, isOutput=False)
    wq = nc.declare_dram_parameter("wq", [HID, DQ], BF16, isOutput=False)
    wk = nc.declare_dram_parameter("wk", [HID, D], BF16, isOutput=False)
    wv = nc.declare_dram_parameter("wv", [HID, D], BF16, isOutput=False)
    wo = nc.declare_dram_parameter("wo", [HID, DQ], BF16, isOutput=False)
    cosT = nc.declare_dram_parameter("cosT", [D, T], F32, isOutput=False)
    sinT = nc.declare_dram_parameter("sinT", [D, T], F32, isOutput=False)  # sign-folded
    negtri = nc.declare_dram_parameter("negtri", [128, 128], BF16, isOutput=False)
    ones = nc.declare_dram_parameter("ones", [128, 1], BF16, isOutput=False)
    onesr = nc.declare_dram_parameter("onesr", [1, 128], F32R, isOutput=False)
    ident = nc.declare_dram_parameter("ident", [128, 128], BF16, isOutput=False)
    out = nc.declare_dram_parameter("out", [T, DQ], F32, isOutput=True)

    att_loc = [nc.dram_tensor(f"att_loc{c}", [DQ, 512], BF16) for c in range(TC)]
    att_full = [nc.dram_tensor(f"att_full{c}", [HID, 512], BF16, addr_space="Shared")
                for c in range(TC)]
    dum_in = nc.dram_tensor("dum_in", [1, 512], BF16)
    dum_out = nc.dram_tensor("dum_out", [NC, 512], BF16, addr_space="Shared")

    inv_sqrt_d = 1.0 / math.sqrt(D)

    with tile.TileContext(nc) as tc:
        with tc.tile_pool(name="persist", bufs=1) as pp:
            # persistent SBUF
            qt_sb = [pp.tile([128, T], BF16, tag=f"qt{h}", name=f"qt{h}")
                     for h in range(HQ)]
            kt_sb = pp.tile([128, T], BF16, tag="kt")
            vt_sb = pp.tile([128, T], BF16, tag="vt")        # V transposed [d, t]
            vn_sb = pp.tile([128, T], BF16, tag="vn")        # V natural [t, d] x16 tiles
            cos_sb = pp.tile([128, T], F32, tag="cos")
            sin_sb = pp.tile([128, T], F32, tag="sin")
            ngt_sb = pp.tile([128, 128], BF16, tag="negtri")
            ones_sb = pp.tile([128, 1], BF16, tag="ones")
            onesr_sb = pp.tile([1, 128], F32R, tag="onesr")
            id_sb = pp.tile([128, 128], BF16, tag="ident")
            wo_sb = pp.tile([128, KT * DQ], BF16, tag="wo")

            nc.sync.dma_start(cos_sb[:, :], cosT[:, :])
            nc.sync.dma_start(sin_sb[:, :], sinT[:, :])
            nc.sync.dma_start(ngt_sb[:, :], negtri[:, :])
            nc.sync.dma_start(ones_sb[:, :], ones[:, :])
            nc.sync.dma_start(onesr_sb[:, :], onesr[:, :])
            nc.sync.dma_start(id_sb[:, :], ident[:, :])
            nc.sync.dma_start(
                wo_sb[:, :].rearrange("p (a m) -> p a m", a=KT),
                wo.rearrange("(a p) m -> p a m", p=128))

            # dummy collective: absorb first-use collective setup latency
            # while phase 1 computes
            nc.gpsimd.collective_compute(
                "AllGather", mybir.AluOpType.bypass,
                replica_groups=[list(range(NC))],
                ins=[dum_in[:, :]], outs=[dum_out[:, :]])

            _phase1_qkv(nc, tc, xT, wq, wk, wv,
                        qt_sb, kt_sb, vt_sb, vn_sb, cos_sb, sin_sb, id_sb)

            with (
                tc.tile_pool(name="attn", bufs=4) as ap,
                tc.tile_pool(name="eacc", bufs=2) as ep,
                tc.tile_pool(name="attops", bufs=3, space="PSUM") as sps,
                tc.tile_pool(name="attacc", bufs=1, space="PSUM") as acc_ps,
                tc.tile_pool(name="attout", bufs=3) as aop,
                tc.tile_pool(name="ostrip", bufs=4) as osp,
                tc.tile_pool(name="ops", bufs=1, space="PSUM") as ops,
                tc.tile_pool(name="oout", bufs=2) as oop,
            ):
                def ph2(qc, fillers=()):
                    _phase2_chunk(nc, tc, ap, ep, sps, acc_ps, aop,
                                  qt_sb, kt_sb, vn_sb, ngt_sb, id_sb,
                                  ones_sb, onesr_sb, att_loc[qc], qc,
                                  inv_sqrt_d, fillers)

                def gather(qc):
                    nc.gpsimd.collective_compute(
                        "AllGather",
                        mybir.AluOpType.bypass,
                        replica_groups=[list(range(NC))],
                        ins=[att_loc[qc][:, :]],
                        outs=[att_full[qc][:, :]],
                    )

                strips = {}

                def load_strips(qc):
                    """4 per-t-tile strips of chunk qc, on the sync queue.
                    Called at the start of ph2(qc+2), by which time AG(qc)
                    completed, so the queue never blocks on the gather."""
                    att_r = att_full[qc].rearrange("(a p) t -> p a t", p=128)
                    sts = []
                    for tt in range(4):
                        st = osp.tile([128, KT * 128], BF16, tag="strip")
                        nc.sync.dma_start(
                            st[:, :].rearrange("p (a f) -> p a f", a=KT),
                            att_r[:, :, tt * 128:(tt + 1) * 128])
                        sts.append(st)
                    strips[qc] = sts

                def ph3_fillers(qc):
                    """8 closures, each emitting half an o_proj psum tile."""
                    fs = []
                    for tt in range(4):
                        holder = {}

                        def first(tt=tt, holder=holder):
                            holder["ps"] = _ph3_half(
                                nc, ops, strips[qc][tt], wo_sb, 0, None)

                        def second(tt=tt, holder=holder, qc=qc):
                            o_ps = _ph3_half(
                                nc, ops, strips[qc][tt], wo_sb, 1, holder["ps"])
                            ot = oop.tile([128, 512], F32, tag="ot")
                            nc.scalar.copy(ot[:, :], o_ps[:, :])
                            row = (qc * 4 + tt) * 128
                            nc.sync.dma_start(out[row:row + 128, :], ot[:, :])
                        fs += [first, second]
                    return fs

                ph2(0)
                gather(0)
                ph2(1)
                gather(1)
                load_strips(0)
                ph2(2, fillers=ph3_fillers(0))
                gather(2)
                load_strips(1)
                ph2(3, fillers=ph3_fillers(1))
                gather(3)
                load_strips(2)
                for f in ph3_fillers(2):
                    f()
                load_strips(3)
                for f in ph3_fillers(3):
                    f()

    nc.compile()
    return nc


def _ph3_half(nc, ops, strip, wo_sb, half, o_ps):
    """Emit 16 of the 32 accumulating o_proj matmuls for one t-tile strip."""
    if o_ps is None:
        o_ps = ops.tile([128, 512], F32, tag="ops")
    for k2 in range(half * 16, half * 16 + 16):
        nc.tensor.matmul(
            o_ps[:, :],
            strip[:, k2 * 128:(k2 + 1) * 128],
            wo_sb[:, k2 * DQ:(k2 + 1) * DQ],
            start=(k2 == 0), stop=(k2 == KT - 1),
            skip_group_check=True)
    return o_ps


def _phase1_qkv(nc, tc, xT, wq, wk, wv,
                qt_sb, kt_sb, vt_sb, vn_sb, cos_sb, sin_sb, id_sb):
    with tc.tile_pool(name="wqkv", bufs=1) as wp:
        wq_sb = wp.tile([128, KT * DQ], BF16, tag="wq")
        wk_sb = wp.tile([128, KT * D], BF16, tag="wk")
        wv_sb = wp.tile([128, KT * D], BF16, tag="wv")
        nc.sync.dma_start(
            wq_sb[:, :].rearrange("p (a m) -> p a m", a=KT),
            wq.rearrange("(a p) m -> p a m", p=128))
        nc.sync.dma_start(
            wk_sb[:, :].rearrange("p (a m) -> p a m", a=KT),
            wk.rearrange("(a p) m -> p a m", p=128))
        nc.sync.dma_start(
            wv_sb[:, :].rearrange("p (a m) -> p a m", a=KT),
            wv.rearrange("(a p) m -> p a m", p=128))

        with (
            tc.tile_pool(name="xrhs", bufs=4) as xp,
            tc.tile_pool(name="qkvps", bufs=1, space="PSUM") as qps,
            tc.tile_pool(name="ropem", bufs=5) as rpm,
            tc.tile_pool(name="ropes", bufs=2) as rps,
            tc.tile_pool(name="vtp", bufs=2, space="PSUM") as vps,
        ):
            for tcn in range(TC):
                ts = tcn * 512
                pq = [qps.tile([128, 512], F32, tag=f"pq{h}", name=f"pq{h}")
                      for h in range(HQ)]
                pk = qps.tile([128, 512], F32, tag="pk")
                pv = qps.tile([128, 512], F32, tag="pv")
                for k in range(KT):
                    xt = xp.tile([128, 512], BF16, tag="xt")
                    nc.sync.dma_start(
                        xt[:, :], xT[k * 128:(k + 1) * 128, ts:ts + 512])
                    for h in range(HQ):
                        nc.tensor.matmul(
                            pq[h][:, :],
                            wq_sb[:, k * DQ + h * 128: k * DQ + (h + 1) * 128],
                            xt[:, :],
                            start=(k == 0), stop=(k == KT - 1),
                        )
                    nc.tensor.matmul(
                        pk[:, :], wk_sb[:, k * D:(k + 1) * D], xt[:, :],
                        start=(k == 0), stop=(k == KT - 1))
                    nc.tensor.matmul(
                        pv[:, :], wv_sb[:, k * D:(k + 1) * D], xt[:, :],
                        start=(k == 0), stop=(k == KT - 1))

                # epilogue. Free the 6 PSUM banks ASAP: per target the DVE
                # cos-mult and ACT copy lead; shuffle/sin/add trail.
                nc.scalar.copy(vt_sb[:, ts:ts + 512], pv[:, :])
                qc_ts, qn_ts = [], []
                for h in range(HQ + 1):
                    src = pq[h] if h < HQ else pk
                    qc_t = rpm.tile([128, 512], F32, tag="qcos")
                    nc.vector.tensor_tensor(
                        qc_t[:, :], src[:, :], cos_sb[:, ts:ts + 512],
                        op=mybir.AluOpType.mult)
                    qn_t = rpm.tile([128, 512], F32, tag="qnat")
                    nc.scalar.copy(qn_t[:, :], src[:, :])
                    qc_ts.append(qc_t)
                    qn_ts.append(qn_t)

                # V transpose for this chunk fills the PE boundary bubble
                for t4 in range(4):
                    tb = ts + t4 * 128
                    vp = vps.tile([128, 128], BF16, tag="vtp")
                    nc.tensor.transpose(
                        vp[:, :], vt_sb[:, tb:tb + 128], id_sb[:, :])
                    nc.scalar.copy(vn_sb[:, tb:tb + 128], vp[:, :])

                for h in range(HQ + 1):
                    dst = qt_sb[h] if h < HQ else kt_sb
                    sh_t = rps.tile([128, 512], F32, tag="qshuf")
                    nc.gpsimd.dma_start(sh_t[0:64, :], qn_ts[h][64:128, :])
                    nc.gpsimd.dma_start(sh_t[64:128, :], qn_ts[h][0:64, :])
                    ss_t = rps.tile([128, 512], F32, tag="qsin")
                    nc.vector.tensor_tensor(
                        ss_t[:, :], sh_t[:, :], sin_sb[:, ts:ts + 512],
                        op=mybir.AluOpType.mult)
                    nc.vector.tensor_tensor(
                        dst[:, ts:ts + 512], qc_ts[h][:, :], ss_t[:, :],
                        op=mybir.AluOpType.add)


def _phase2_chunk(nc, tc, ap, ep, sps, acc_ps, aop,
                  qt_sb, kt_sb, vn_sb, ngt_sb, id_sb, ones_sb, onesr_sb,
                  att_loc_qc, qc, inv_sqrt_d, fillers):
    """Attention for q-chunk qc (512 queries), all HQ heads together.

    fillers: list of closures (o_proj work of an older chunk) interleaved
    into the kt loop to keep the PE dense while the S->exp->PV chains drain.
    """
    qs = qc * 512
    n_kt = 4 * (qc + 1)
    o_ps = [acc_ps.tile([128, 512], F32, tag=f"opv{h}", name=f"opv{h}")
            for h in range(HQ)]
    e_acc = [ep.tile([128, 512], F32, tag=f"eacc{h}", name=f"eacc{h}")
             for h in range(HQ)]

    def ncols(kt):
        m = kt - 4 * qc
        return 512 - 128 * m if m >= 0 else 512

    def s_exp(kt, h):
        """S matmul (+ additive -3000 triangle on the PE for the leading
        128 cols of diagonal tiles) + exp + denominator accumulate split
        across DVE (h 0-1) and gpsimd (h 2-3)."""
        n = ncols(kt)
        off = 512 - n
        diag = kt - 4 * qc >= 0
        s_ps = sps.tile([128, 512], F32, tag="st")
        nc.tensor.matmul(
            s_ps[:, 0:n],
            kt_sb[:, kt * 128:(kt + 1) * 128],
            qt_sb[h][:, qs + off:qs + 512],
            start=True, stop=not diag, skip_group_check=True)
        if diag:
            nc.tensor.matmul(
                s_ps[:, 0:128], id_sb[:, :], ngt_sb[:, :],
                start=False, stop=True, skip_group_check=True)
        e_t = ap.tile([128, 512], BF16, tag=f"et{h}", name=f"et{h}")
        nc.scalar.activation(
            e_t[:, 0:n], s_ps[:, 0:n],
            mybir.ActivationFunctionType.Exp,
            scale=inv_sqrt_d)
        eng = nc.vector if h < 2 else nc.gpsimd
        if kt == 0:
            eng.tensor_scalar_add(e_acc[h][:, :], e_t[:, :], 0.0)
        else:
            eng.tensor_tensor(
                e_acc[h][:, off:512], e_acc[h][:, off:512], e_t[:, 0:n],
                op=mybir.AluOpType.add)
        return e_t

    def pv(kt, e_ts):
        n = ncols(kt)
        off = 512 - n
        for h in range(HQ):
            nc.tensor.matmul(
                o_ps[h][:, off:512],
                vn_sb[:, kt * 128:(kt + 1) * 128],
                e_ts[h][:, 0:n],
                start=(kt == 0), stop=(kt == n_kt - 1),
                skip_group_check=True)

    fillers = list(fillers)
    nfill = len(fillers)
    fill_done = 0

    prev = None
    for kt in range(n_kt):
        cur = [s_exp(kt, h) for h in range(HQ)]
        if prev is not None:
            pv(kt - 1, prev)
        prev = cur
        # interleave o_proj filler work of an older chunk
        want = nfill * (kt + 1) // n_kt
        while fill_done < want:
            fillers[fill_done]()
            fill_done += 1
    pv(n_kt - 1, prev)

    # denominator: bf16 convert on DVE, then 4 col-tiled bf16 matmuls
    # partition-reduce into one PSUM bank
    den_ps = sps.tile([128, 512], F32, tag="st")
    for h in range(HQ):
        e_bf = ap.tile([128, 512], BF16, tag=f"ebf{h}", name=f"ebf{h}")
        eng = nc.vector if h < 2 else nc.gpsimd
        eng.tensor_scalar_add(e_bf[:, :], e_acc[h][:, :], 0.0)
        nc.tensor.matmul(
            den_ps[32 * h:32 * h + 1, :],
            ones_sb[:, :], e_bf[:, :],
            start=True, stop=True,
            skip_group_check=True, tile_position=(0, 32 * h))

    # normalize + store transposed attention chunk.
    # all reciprocals first so the PE rb matmuls run gapless, then the
    # per-head multiply/store chains.
    rb_sbs = []
    for h in range(HQ):
        rc_t = aop.tile([1, 512], F32R, tag="recip")
        with nc.allow_low_precision(reason="f32r is bitwise f32"):
            nc.vector.reciprocal(rc_t[:, :], den_ps[32 * h:32 * h + 1, :])
        rb_ps = sps.tile([128, 512], F32, tag="st")
        nc.tensor.matmul(
            rb_ps[:, :], onesr_sb[:, :], rc_t[:, :],
            start=True, stop=True, skip_group_check=True)
        rb_sb = aop.tile([128, 512], BF16, tag=f"rbb{h}", name=f"rbb{h}")
        nc.scalar.copy(rb_sb[:, :], rb_ps[:, :])
        rb_sbs.append(rb_sb)
    for h in range(HQ):
        at_t = aop.tile([128, 512], BF16, tag="attT")
        nc.vector.tensor_tensor(
            at_t[:, :], o_ps[h][:, :], rb_sbs[h][:, :],
            op=mybir.AluOpType.mult)
        nc.sync.dma_start(
            att_loc_qc[h * 128:(h + 1) * 128, :], at_t[:, :])


def _host_consts():
    # rope tables, transposed + sign-folded
    inv = 1.0 / (ROPE_BASE ** (np.arange(0, D, 2, dtype=np.float32) / D))
    t = np.arange(T, dtype=np.float32)
    f = np.outer(t, inv)
    e = np.concatenate([f, f], axis=-1)
    cos = np.cos(e).astype(np.float32)
    sin = np.sin(e).astype(np.float32)
    sgn = np.where(np.arange(D) < D // 2, -1.0, 1.0).astype(np.float32)
    cosT = np.ascontiguousarray(cos.T)
    sinT = np.ascontiguousarray((sin * sgn).T)
    # additive causal mask for the leading 128 cols of a diagonal tile:
    # 0 where kept (j >= i), -3000 where masked (exp(scale*(s-3000)) == 0)
    p = np.arange(128)[:, None]
    fr = np.arange(128)[None, :]
    negtri = np.where(fr - p >= 0, 0.0, -3000.0).astype(NPBF16)
    ones = np.ones((128, 1), NPBF16)
    onesr = np.ones((1, 128), np.float32)
    ident = np.eye(128, dtype=NPBF16)
    return cosT, sinT, negtri, ones, onesr, ident


def kernel(x, wq, wk, wv, wo, mask=None, **_ignored):
    x = np.asarray(x, dtype=np.float32)
    wq = np.asarray(wq, dtype=np.float32)
    wk = np.asarray(wk, dtype=np.float32)
    wv = np.asarray(wv, dtype=np.float32)
    wo = np.asarray(wo, dtype=np.float32)
    B = x.shape[0]
    xT = np.ascontiguousarray(x.reshape(T, HID).T).astype(NPBF16)   # [HID, T]
    cosT, sinT, negtri, ones, onesr, ident = _host_consts()

    if "nc" not in _BUILD_CACHE:
        _BUILD_CACHE["nc"] = _build_nc()
    nc = _BUILD_CACHE["nc"]

    in_maps = []
    for i in range(NC):
        in_maps.append({
            "xT": xT,
            "wq": np.ascontiguousarray(wq[:, i * DQ:(i + 1) * DQ]).astype(NPBF16),
            "wk": np.ascontiguousarray(wk[:, i * D:(i + 1) * D]).astype(NPBF16),
            "wv": np.ascontiguousarray(wv[:, i * D:(i + 1) * D]).astype(NPBF16),
            "wo": np.ascontiguousarray(wo[:, i * DQ:(i + 1) * DQ]).astype(NPBF16),
            "cosT": cosT, "sinT": sinT, "negtri": negtri, "ones": ones,
            "onesr": onesr, "ident": ident,
        })

    res = run_bass_kernel_spmd(nc, in_maps, core_ids=list(range(NC)), **RUN_KWARGS)
    _BUILD_CACHE["last_res"] = res
    out = np.concatenate([res.results[i]["out"] for i in range(NC)], axis=1)
    return out.reshape(B, T, HID)


if __name__ == "__main__":
    rng = np.random.default_rng(0)
    s = 1.0 / math.sqrt(HID)
    x = rng.standard_normal((1, T, HID), dtype=np.float32)
    wq_ = rng.standard_normal((HID, H * D), dtype=np.float32) * s
    wk_ = rng.standard_normal((HID, KV * D), dtype=np.float32) * s
    wv_ = rng.standard_normal((HID, KV * D), dtype=np.float32) * s
    wo_ = rng.standard_normal((HID, H * D), dtype=np.float32) * s
    o = kernel(x, wq_, wk_, wv_, wo_, None)
    print("out", o.shape, o.dtype, float(np.abs(o).mean()))


# revision 16
# speedup vs baseline: 1.0710x; 1.0710x over previous
"""GQA (H=32, KV=8, D=128, T=2048, hid=4096) fp32 causal attention + RoPE,
tensor-parallel over heads across 8 NeuronCores.

v3: bf16 matmul datapath (fp32 accumulate) with a fully software-pipelined
schedule tuned from perfetto traces:
  - Phase 1 (QKV+RoPE): per-t-chunk epilogue reordered so the 6 PSUM
    accumulator banks free within ~1us of the last matmul (cos-mult on DVE +
    copy on ACT lead), V-transposes moved inside the chunk loop to fill the
    PE boundary bubble, RoPE shuffle DMAs moved to the gpsimd queue so the
    sync queue keeps streaming x tiles.
  - Phase 2 (attention): S matmuls + exp (ACT) + causal tri-mask (DVE, only
    the 128 leading cols of diagonal tiles) + PV, with the softmax
    denominator accumulated on DVE (f32r) and reduced across partitions by 4
    col-tiled f32r matmuls per chunk. Diagonal tiles column-restricted
    (N = 512-128m).
  - Per q-chunk AllGather [512,512]bf16 -> [4096,512], first-use latency
    absorbed by a dummy collective during phase 1.
  - Phase 3 (o_proj): whole-chunk activation strip DMA'd from shared HBM on
    the gpsimd queue; matmul tiles interleaved INTO phase 2's kt loop two
    chunks later (gather latency pipeline depth 2) to keep the PE dense.
Host concatenates the 8 column slices.
"""

import math
import numpy as np
import ml_dtypes

import concourse.bass as bass
import concourse.mybir as mybir
import concourse.tile as tile
from concourse import bacc
from concourse.bass_utils import run_bass_kernel_spmd

T = 2048
HID = 4096
H = 32
KV = 8
D = 128
NC = 8
HQ = H // NC          # 4 query heads per core
DQ = HQ * D           # 512
KT = HID // 128       # 32 contraction tiles
TC = T // 512         # 4 t-chunks
ROPE_BASE = 10000.0

BF16 = mybir.dt.bfloat16
F32 = mybir.dt.float32
F32R = mybir.dt.float32r
NPBF16 = ml_dtypes.bfloat16

_BUILD_CACHE = {}
RUN_KWARGS = {}  # test harness hook (e.g. {"trace": True})


def _build_nc():
    nc = bacc.Bacc(None, target_bir_lowering=False, num_devices=NC)

    xT = nc.declare_dram_parameter("xT", [HID, T], BF16, isOutput=False)
    wq = nc.declare_dram_parameter("wq", [HID, DQ], BF16, isOutput=False)
    wk = nc.declare_dram_parameter("wk", [HID, D], BF16, isOutput=False)
    wv = nc.declare_dram_parameter("wv", [HID, D], BF16, isOutput=False)
    wo = nc.declare_dram_parameter("wo", [HID, DQ], BF16, isOutput=False)
    cosT = nc.declare_dram_parameter("cosT", [D, T], F32, isOutput=False)
    sinT = nc.declare_dram_parameter("sinT", [D, T], F32, isOutput=False)  # sign-folded
    negtri = nc.declare_dram_parameter("negtri", [128, 128], BF16, isOutput=False)
    ones = nc.declare_dram_parameter("ones", [128, 1], BF16, isOutput=False)
    onesr = nc.declare_dram_parameter("onesr", [1, 128], F32R, isOutput=False)
    ident = nc.declare_dram_parameter("ident", [128, 128], BF16, isOutput=False)
    out = nc.declare_dram_parameter("out", [T, DQ], F32, isOutput=True)

    att_loc = [nc.dram_tensor(f"att_loc{c}", [DQ, 512], BF16) for c in range(TC)]
    att_full = [nc.dram_tensor(f"att_full{c}", [HID, 512], BF16, addr_space="Shared")
                for c in range(TC)]
    dum_in = nc.dram_tensor("dum_in", [1, 512], BF16)
    dum_out = nc.dram_tensor("dum_out", [NC, 512], BF16, addr_space="Shared")

    inv_sqrt_d = 1.0 / math.sqrt(D)

    with tile.TileContext(nc) as tc:
        with tc.tile_pool(name="persist", bufs=1) as pp:
            # persistent SBUF
            qt_sb = [pp.tile([128, T], BF16, tag=f"qt{h}", name=f"qt{h}")
                     for h in range(HQ)]
            kt_sb = pp.tile([128, T], BF16, tag="kt")
            vt_sb = pp.tile([128, T], BF16, tag="vt")        # V transposed [d, t]
            vn_sb = pp.tile([128, T], BF16, tag="vn")        # V natural [t, d] x16 tiles
            cos_sb = pp.tile([128, T], F32, tag="cos")
            sin_sb = pp.tile([128, T], F32, tag="sin")
            ngt_sb = pp.tile([128, 128], BF16, tag="negtri")
            ones_sb = pp.tile([128, 1], BF16, tag="ones")
            onesr_sb = pp.tile([1, 128], F32R, tag="onesr")
            id_sb = pp.tile([128, 128], BF16, tag="ident")
            wo_sb = pp.tile([128, KT * DQ], BF16, tag="wo")

            nc.sync.dma_start(cos_sb[:, :], cosT[:, :])
            nc.sync.dma_start(sin_sb[:, :], sinT[:, :])
            nc.sync.dma_start(ngt_sb[:, :], negtri[:, :])
            nc.sync.dma_start(ones_sb[:, :], ones[:, :])
            nc.sync.dma_start(onesr_sb[:, :], onesr[:, :])
            nc.sync.dma_start(id_sb[:, :], ident[:, :])
            nc.sync.dma_start(
                wo_sb[:, :].rearrange("p (a m) -> p a m", a=KT),
                wo.rearrange("(a p) m -> p a m", p=128))

            # dummy collective: absorb first-use collective setup latency
            # while phase 1 computes
            nc.gpsimd.collective_compute(
                "AllGather", mybir.AluOpType.bypass,
                replica_groups=[list(range(NC))],
                ins=[dum_in[:, :]], outs=[dum_out[:, :]])

            _phase1_qkv(nc, tc, xT, wq, wk, wv,
                        qt_sb, kt_sb, vt_sb, vn_sb, cos_sb, sin_sb, id_sb)

            with (
                tc.tile_pool(name="attn", bufs=6) as ap,
                tc.tile_pool(name="eacc", bufs=2) as ep,
                tc.tile_pool(name="attops", bufs=3, space="PSUM") as sps,
                tc.tile_pool(name="attacc", bufs=1, space="PSUM") as acc_ps,
                tc.tile_pool(name="attout", bufs=3) as aop,
                tc.tile_pool(name="ostrip", bufs=4) as osp,
                tc.tile_pool(name="ops", bufs=1, space="PSUM") as ops,
                tc.tile_pool(name="oout", bufs=2) as oop,
            ):
                def ph2(qc, fillers=()):
                    _phase2_chunk(nc, tc, ap, ep, sps, acc_ps, aop,
                                  qt_sb, kt_sb, vn_sb, ngt_sb, id_sb,
                                  ones_sb, onesr_sb, att_loc[qc], qc,
                                  inv_sqrt_d, fillers)

                def gather(qc):
                    nc.gpsimd.collective_compute(
                        "AllGather",
                        mybir.AluOpType.bypass,
                        replica_groups=[list(range(NC))],
                        ins=[att_loc[qc][:, :]],
                        outs=[att_full[qc][:, :]],
                    )

                strips = {}

                def load_strips(qc, spread=False):
                    """4 per-t-tile strips of chunk qc. Normally on the sync
                    queue, called at the start of ph2(qc+2) by which time
                    AG(qc) completed so the queue never blocks. With spread
                    (endgame, other queues idle) one DMA per engine so all
                    four strips transfer in parallel."""
                    att_r = att_full[qc].rearrange("(a p) t -> p a t", p=128)
                    engs = ([nc.sync, nc.gpsimd, nc.scalar, nc.sync]
                            if spread else [nc.sync] * 4)
                    sts = []
                    for tt in range(4):
                        st = osp.tile([128, KT * 128], BF16, tag="strip")
                        engs[tt].dma_start(
                            st[:, :].rearrange("p (a f) -> p a f", a=KT),
                            att_r[:, :, tt * 128:(tt + 1) * 128])
                        sts.append(st)
                    strips[qc] = sts

                def ph3_fillers(qc):
                    """8 closures, each emitting half an o_proj psum tile."""
                    fs = []
                    for tt in range(4):
                        holder = {}

                        def first(tt=tt, holder=holder):
                            holder["ps"] = _ph3_half(
                                nc, ops, strips[qc][tt], wo_sb, 0, None)

                        def second(tt=tt, holder=holder, qc=qc):
                            o_ps = _ph3_half(
                                nc, ops, strips[qc][tt], wo_sb, 1, holder["ps"])
                            ot = oop.tile([128, 512], F32, tag="ot")
                            nc.scalar.copy(ot[:, :], o_ps[:, :])
                            row = (qc * 4 + tt) * 128
                            nc.sync.dma_start(out[row:row + 128, :], ot[:, :])
                        fs += [first, second]
                    return fs

                ph2(0)
                gather(0)
                ph2(1)
                gather(1)
                load_strips(0)
                ph2(2, fillers=ph3_fillers(0))
                gather(2)
                load_strips(1)
                ph2(3, fillers=ph3_fillers(1))
                gather(3)
                load_strips(2, spread=True)
                for f in ph3_fillers(2):
                    f()
                load_strips(3, spread=True)
                for f in ph3_fillers(3):
                    f()

    nc.compile()
    return nc


def _ph3_half(nc, ops, strip, wo_sb, half, o_ps):
    """Emit 16 of the 32 accumulating o_proj matmuls for one t-tile strip."""
    if o_ps is None:
        o_ps = ops.tile([128, 512], F32, tag="ops")
    for k2 in range(half * 16, half * 16 + 16):
        nc.tensor.matmul(
            o_ps[:, :],
            strip[:, k2 * 128:(k2 + 1) * 128],
            wo_sb[:, k2 * DQ:(k2 + 1) * DQ],
            start=(k2 == 0), stop=(k2 == KT - 1),
            skip_group_check=True)
    return o_ps


def _phase1_qkv(nc, tc, xT, wq, wk, wv,
                qt_sb, kt_sb, vt_sb, vn_sb, cos_sb, sin_sb, id_sb):
    with tc.tile_pool(name="wqkv", bufs=1) as wp:
        wq_sb = wp.tile([128, KT * DQ], BF16, tag="wq")
        wk_sb = wp.tile([128, KT * D], BF16, tag="wk")
        wv_sb = wp.tile([128, KT * D], BF16, tag="wv")
        nc.sync.dma_start(
            wq_sb[:, :].rearrange("p (a m) -> p a m", a=KT),
            wq.rearrange("(a p) m -> p a m", p=128))
        nc.sync.dma_start(
            wk_sb[:, :].rearrange("p (a m) -> p a m", a=KT),
            wk.rearrange("(a p) m -> p a m", p=128))
        nc.sync.dma_start(
            wv_sb[:, :].rearrange("p (a m) -> p a m", a=KT),
            wv.rearrange("(a p) m -> p a m", p=128))

        with (
            tc.tile_pool(name="xrhs", bufs=4) as xp,
            tc.tile_pool(name="qkvps", bufs=1, space="PSUM") as qps,
            tc.tile_pool(name="ropem", bufs=5) as rpm,
            tc.tile_pool(name="ropes", bufs=2) as rps,
            tc.tile_pool(name="vtp", bufs=2, space="PSUM") as vps,
        ):
            for tcn in range(TC):
                ts = tcn * 512
                pq = [qps.tile([128, 512], F32, tag=f"pq{h}", name=f"pq{h}")
                      for h in range(HQ)]
                pk = qps.tile([128, 512], F32, tag="pk")
                pv = qps.tile([128, 512], F32, tag="pv")
                for k in range(KT):
                    xt = xp.tile([128, 512], BF16, tag="xt")
                    nc.sync.dma_start(
                        xt[:, :], xT[k * 128:(k + 1) * 128, ts:ts + 512])
                    for h in range(HQ):
                        nc.tensor.matmul(
                            pq[h][:, :],
                            wq_sb[:, k * DQ + h * 128: k * DQ + (h + 1) * 128],
                            xt[:, :],
                            start=(k == 0), stop=(k == KT - 1),
                        )
                    nc.tensor.matmul(
                        pk[:, :], wk_sb[:, k * D:(k + 1) * D], xt[:, :],
                        start=(k == 0), stop=(k == KT - 1))
                    nc.tensor.matmul(
                        pv[:, :], wv_sb[:, k * D:(k + 1) * D], xt[:, :],
                        start=(k == 0), stop=(k == KT - 1))

                # epilogue. Free the 6 PSUM banks ASAP: per target the DVE
                # cos-mult and ACT copy lead; shuffle/sin/add trail.
                nc.scalar.copy(vt_sb[:, ts:ts + 512], pv[:, :])
                qc_ts, qn_ts = [], []
                for h in range(HQ + 1):
                    src = pq[h] if h < HQ else pk
                    qc_t = rpm.tile([128, 512], F32, tag="qcos")
                    nc.vector.tensor_tensor(
                        qc_t[:, :], src[:, :], cos_sb[:, ts:ts + 512],
                        op=mybir.AluOpType.mult)
                    qn_t = rpm.tile([128, 512], F32, tag="qnat")
                    nc.scalar.copy(qn_t[:, :], src[:, :])
                    qc_ts.append(qc_t)
                    qn_ts.append(qn_t)

                # V transpose for this chunk fills the PE boundary bubble
                for t4 in range(4):
                    tb = ts + t4 * 128
                    vp = vps.tile([128, 128], BF16, tag="vtp")
                    nc.tensor.transpose(
                        vp[:, :], vt_sb[:, tb:tb + 128], id_sb[:, :])
                    nc.scalar.copy(vn_sb[:, tb:tb + 128], vp[:, :])

                eng = nc.gpsimd if tcn == TC - 1 else nc.vector
                for h in range(HQ + 1):
                    dst = qt_sb[h] if h < HQ else kt_sb
                    sh_t = rps.tile([128, 512], F32, tag="qshuf")
                    nc.gpsimd.dma_start(sh_t[0:64, :], qn_ts[h][64:128, :])
                    nc.gpsimd.dma_start(sh_t[64:128, :], qn_ts[h][0:64, :])
                    ss_t = rps.tile([128, 512], F32, tag="qsin")
                    eng.tensor_tensor(
                        ss_t[:, :], sh_t[:, :], sin_sb[:, ts:ts + 512],
                        op=mybir.AluOpType.mult)
                    eng.tensor_tensor(
                        dst[:, ts:ts + 512], qc_ts[h][:, :], ss_t[:, :],
                        op=mybir.AluOpType.add)


def _phase2_chunk(nc, tc, ap, ep, sps, acc_ps, aop,
                  qt_sb, kt_sb, vn_sb, ngt_sb, id_sb, ones_sb, onesr_sb,
                  att_loc_qc, qc, inv_sqrt_d, fillers):
    """Attention for q-chunk qc (512 queries), all HQ heads together.

    fillers: list of closures (o_proj work of an older chunk) interleaved
    into the kt loop to keep the PE dense while the S->exp->PV chains drain.
    """
    qs = qc * 512
    n_kt = 4 * (qc + 1)
    o_ps = [acc_ps.tile([128, 512], F32, tag=f"opv{h}", name=f"opv{h}")
            for h in range(HQ)]
    e_acc = [ep.tile([128, 512], F32, tag=f"eacc{h}", name=f"eacc{h}")
             for h in range(HQ)]

    def ncols(kt):
        m = kt - 4 * qc
        return 512 - 128 * m if m >= 0 else 512

    def s_exp(kt, h):
        """S matmul (+ additive -3000 triangle on the PE for the leading
        128 cols of diagonal tiles) + exp + denominator accumulate split
        across DVE (h 0-1) and gpsimd (h 2-3)."""
        n = ncols(kt)
        off = 512 - n
        diag = kt - 4 * qc >= 0
        s_ps = sps.tile([128, 512], F32, tag="st")
        nc.tensor.matmul(
            s_ps[:, 0:n],
            kt_sb[:, kt * 128:(kt + 1) * 128],
            qt_sb[h][:, qs + off:qs + 512],
            start=True, stop=not diag, skip_group_check=True)
        if diag:
            nc.tensor.matmul(
                s_ps[:, 0:128], id_sb[:, :], ngt_sb[:, :],
                start=False, stop=True, skip_group_check=True)
        e_t = ap.tile([128, 512], BF16, tag=f"et{h}", name=f"et{h}")
        nc.scalar.activation(
            e_t[:, 0:n], s_ps[:, 0:n],
            mybir.ActivationFunctionType.Exp,
            scale=inv_sqrt_d)
        eng = nc.vector
        if kt == 0:
            eng.tensor_scalar_add(e_acc[h][:, :], e_t[:, :], 0.0)
        else:
            eng.tensor_tensor(
                e_acc[h][:, off:512], e_acc[h][:, off:512], e_t[:, 0:n],
                op=mybir.AluOpType.add)
        return e_t

    def pv(kt, e_ts):
        n = ncols(kt)
        off = 512 - n
        for h in range(HQ):
            nc.tensor.matmul(
                o_ps[h][:, off:512],
                vn_sb[:, kt * 128:(kt + 1) * 128],
                e_ts[h][:, 0:n],
                start=(kt == 0), stop=(kt == n_kt - 1),
                skip_group_check=True)

    fillers = list(fillers)
    nfill = len(fillers)
    fill_done = 0

    prev = None
    for kt in range(n_kt):
        cur = [s_exp(kt, h) for h in range(HQ)]
        if prev is not None:
            pv(kt - 1, prev)
        prev = cur
        # interleave o_proj filler work of an older chunk
        want = nfill * (kt + 1) // n_kt
        while fill_done < want:
            fillers[fill_done]()
            fill_done += 1
    pv(n_kt - 1, prev)

    # denominator: bf16 convert on DVE, then 4 col-tiled bf16 matmuls
    # partition-reduce into one PSUM bank
    den_ps = sps.tile([128, 512], F32, tag="st")
    for h in range(HQ):
        e_bf = ap.tile([128, 512], BF16, tag=f"ebf{h}", name=f"ebf{h}")
        nc.vector.tensor_scalar_add(e_bf[:, :], e_acc[h][:, :], 0.0)
        nc.tensor.matmul(
            den_ps[32 * h:32 * h + 1, :],
            ones_sb[:, :], e_bf[:, :],
            start=True, stop=True,
            skip_group_check=True, tile_position=(0, 32 * h))

    # normalize + store transposed attention chunk.
    # all reciprocals first so the PE rb matmuls run gapless, then the
    # per-head multiply/store chains.
    rb_sbs = []
    for h in range(HQ):
        rc_t = aop.tile([1, 512], F32R, tag="recip")
        with nc.allow_low_precision(reason="f32r is bitwise f32"):
            nc.vector.reciprocal(rc_t[:, :], den_ps[32 * h:32 * h + 1, :])
        rb_ps = sps.tile([128, 512], F32, tag="st")
        nc.tensor.matmul(
            rb_ps[:, :], onesr_sb[:, :], rc_t[:, :],
            start=True, stop=True, skip_group_check=True)
        rb_sb = aop.tile([128, 512], BF16, tag=f"rbb{h}", name=f"rbb{h}")
        nc.scalar.copy(rb_sb[:, :], rb_ps[:, :])
        rb_sbs.append(rb_sb)
    for h in range(HQ):
        at_t = aop.tile([128, 512], BF16, tag="attT")
        nc.vector.tensor_tensor(
            at_t[:, :], o_ps[h][:, :], rb_sbs[h][:, :],
            op=mybir.AluOpType.mult)
        nc.sync.dma_start(
            att_loc_qc[h * 128:(h + 1) * 128, :], at_t[:, :])


def _host_consts():
    # rope tables, transposed + sign-folded
    inv = 1.0 / (ROPE_BASE ** (np.arange(0, D, 2, dtype=np.float32) / D))
    t = np.arange(T, dtype=np.float32)
    f = np.outer(t, inv)
    e = np.concatenate([f, f], axis=-1)
    cos = np.cos(e).astype(np.float32)
    sin = np.sin(e).astype(np.float32)
    sgn = np.where(np.arange(D) < D // 2, -1.0, 1.0).astype(np.float32)
    cosT = np.ascontiguousarray(cos.T)
    sinT = np.ascontiguousarray((sin * sgn).T)
    # additive causal mask for the leading 128 cols of a diagonal tile:
    # 0 where kept (j >= i), -3000 where masked (exp(scale*(s-3000)) == 0)
    p = np.arange(128)[:, None]
    fr = np.arange(128)[None, :]
    negtri = np.where(fr - p >= 0, 0.0, -3000.0).astype(NPBF16)
    ones = np.ones((128, 1), NPBF16)
    onesr = np.ones((1, 128), np.float32)
    ident = np.eye(128, dtype=NPBF16)
    return cosT, sinT, negtri, ones, onesr, ident


def kernel(x, wq, wk, wv, wo, mask=None, **_ignored):
    x = np.asarray(x, dtype=np.float32)
    wq = np.asarray(wq, dtype=np.float32)
    wk = np.asarray(wk, dtype=np.float32)
    wv = np.asarray(wv, dtype=np.float32)
    wo = np.asarray(wo, dtype=np.float32)
    B = x.shape[0]
    xT = np.ascontiguousarray(x.reshape(T, HID).T).astype(NPBF16)   # [HID, T]
    cosT, sinT, negtri, ones, onesr, ident = _host_consts()

    if "nc" not in _BUILD_CACHE:
        _BUILD_CACHE["nc"] = _build_nc()
    nc = _BUILD_CACHE["nc"]

    in_maps = []
    for i in range(NC):
        in_maps.append({
            "xT": xT,
            "wq": np.ascontiguousarray(wq[:, i * DQ:(i + 1) * DQ]).astype(NPBF16),
            "wk": np.ascontiguousarray(wk[:, i * D:(i + 1) * D]).astype(NPBF16),
            "wv": np.ascontiguousarray(wv[:, i * D:(i + 1) * D]).astype(NPBF16),
            "wo": np.ascontiguousarray(wo[:, i * DQ:(i + 1) * DQ]).astype(NPBF16),
            "cosT": cosT, "sinT": sinT, "negtri": negtri, "ones": ones,
            "onesr": onesr, "ident": ident,
        })

    res = run_bass_kernel_spmd(nc, in_maps, core_ids=list(range(NC)), **RUN_KWARGS)
    _BUILD_CACHE["last_res"] = res
    out = np.concatenate([res.results[i]["out"] for i in range(NC)], axis=1)
    return out.reshape(B, T, HID)


if __name__ == "__main__":
    rng = np.random.default_rng(0)
    s = 1.0 / math.sqrt(HID)
    x = rng.standard_normal((1, T, HID), dtype=np.float32)
    wq_ = rng.standard_normal((HID, H * D), dtype=np.float32) * s
    wk_ = rng.standard_normal((HID, KV * D), dtype=np.float32) * s
    wv_ = rng.standard_normal((HID, KV * D), dtype=np.float32) * s
    wo_ = rng.standard_normal((HID, H * D), dtype=np.float32) * s
    o = kernel(x, wq_, wk_, wv_, wo_, None)
    print("out", o.shape, o.dtype, float(np.abs(o).mean()))


# revision 17
# speedup vs baseline: 1.1351x; 1.0599x over previous
"""GQA (H=32, KV=8, D=128, T=2048, hid=4096) fp32 causal attention + RoPE,
tensor-parallel over heads across 8 NeuronCores.

v3: bf16 matmul datapath (fp32 accumulate) with a fully software-pipelined
schedule tuned from perfetto traces:
  - Phase 1 (QKV+RoPE): per-t-chunk epilogue reordered so the 6 PSUM
    accumulator banks free within ~1us of the last matmul (cos-mult on DVE +
    copy on ACT lead), V-transposes moved inside the chunk loop to fill the
    PE boundary bubble, RoPE shuffle DMAs moved to the gpsimd queue so the
    sync queue keeps streaming x tiles.
  - Phase 2 (attention): S matmuls + exp (ACT) + causal tri-mask (DVE, only
    the 128 leading cols of diagonal tiles) + PV, with the softmax
    denominator accumulated on DVE (f32r) and reduced across partitions by 4
    col-tiled f32r matmuls per chunk. Diagonal tiles column-restricted
    (N = 512-128m).
  - Per q-chunk AllGather [512,512]bf16 -> [4096,512], first-use latency
    absorbed by a dummy collective during phase 1.
  - Phase 3 (o_proj): whole-chunk activation strip DMA'd from shared HBM on
    the gpsimd queue; matmul tiles interleaved INTO phase 2's kt loop two
    chunks later (gather latency pipeline depth 2) to keep the PE dense.
Host concatenates the 8 column slices.
"""

import math
import numpy as np
import ml_dtypes

import concourse.bass as bass
import concourse.mybir as mybir
import concourse.tile as tile
from concourse import bacc
from concourse.bass_utils import run_bass_kernel_spmd

T = 2048
HID = 4096
H = 32
KV = 8
D = 128
NC = 8
HQ = H // NC          # 4 query heads per core
DQ = HQ * D           # 512
KT = HID // 128       # 32 contraction tiles
TC = T // 512         # 4 t-chunks
ROPE_BASE = 10000.0

BF16 = mybir.dt.bfloat16
F32 = mybir.dt.float32
F32R = mybir.dt.float32r
NPBF16 = ml_dtypes.bfloat16

_BUILD_CACHE = {}
RUN_KWARGS = {}  # test harness hook (e.g. {"trace": True})


def _build_nc():
    nc = bacc.Bacc(None, target_bir_lowering=False, num_devices=NC)

    xT = nc.declare_dram_parameter("xT", [HID, T], BF16, isOutput=False)
    wq = nc.declare_dram_parameter("wq", [HID, DQ], BF16, isOutput=False)
    wk = nc.declare_dram_parameter("wk", [HID, D], BF16, isOutput=False)
    wv = nc.declare_dram_parameter("wv", [HID, D], BF16, isOutput=False)
    wo = nc.declare_dram_parameter("wo", [HID, DQ], BF16, isOutput=False)
    cosT = nc.declare_dram_parameter("cosT", [D, T], F32, isOutput=False)
    sinT = nc.declare_dram_parameter("sinT", [D, T], F32, isOutput=False)  # sign-folded
    tri = nc.declare_dram_parameter("tri", [128, 128], BF16, isOutput=False)
    ones = nc.declare_dram_parameter("ones", [128, 1], BF16, isOutput=False)
    onesr = nc.declare_dram_parameter("onesr", [1, 128], F32R, isOutput=False)
    ident = nc.declare_dram_parameter("ident", [128, 128], BF16, isOutput=False)
    out = nc.declare_dram_parameter("out", [T, DQ], F32, isOutput=True)

    att_loc = [nc.dram_tensor(f"att_loc{c}", [DQ, 512], BF16) for c in range(TC)]
    att_full = [nc.dram_tensor(f"att_full{c}", [HID, 512], BF16, addr_space="Shared")
                for c in range(TC)]
    dum_in = nc.dram_tensor("dum_in", [1, 512], BF16)
    dum_out = nc.dram_tensor("dum_out", [NC, 512], BF16, addr_space="Shared")

    inv_sqrt_d = 1.0 / math.sqrt(D)

    with tile.TileContext(nc) as tc:
        with tc.tile_pool(name="persist", bufs=1) as pp:
            # persistent SBUF
            qt_sb = [pp.tile([128, T], BF16, tag=f"qt{h}", name=f"qt{h}")
                     for h in range(HQ)]
            kt_sb = pp.tile([128, T], BF16, tag="kt")
            vt_sb = pp.tile([128, T], BF16, tag="vt")        # V transposed [d, t]
            vn_sb = pp.tile([128, T], BF16, tag="vn")        # V natural [t, d] x16 tiles
            cos_sb = pp.tile([128, T], F32, tag="cos")
            sin_sb = pp.tile([128, T], F32, tag="sin")
            tri_sb = pp.tile([128, 128], BF16, tag="tri")
            ones_sb = pp.tile([128, 1], BF16, tag="ones")
            onesr_sb = pp.tile([1, 128], F32R, tag="onesr")
            id_sb = pp.tile([128, 128], BF16, tag="ident")
            wo_sb = pp.tile([128, KT * DQ], BF16, tag="wo")

            nc.sync.dma_start(cos_sb[:, :], cosT[:, :])
            nc.sync.dma_start(sin_sb[:, :], sinT[:, :])
            nc.sync.dma_start(tri_sb[:, :], tri[:, :])
            nc.sync.dma_start(ones_sb[:, :], ones[:, :])
            nc.sync.dma_start(onesr_sb[:, :], onesr[:, :])
            nc.sync.dma_start(id_sb[:, :], ident[:, :])
            nc.sync.dma_start(
                wo_sb[:, :].rearrange("p (a m) -> p a m", a=KT),
                wo.rearrange("(a p) m -> p a m", p=128))

            # dummy collective: absorb first-use collective setup latency
            # while phase 1 computes
            nc.gpsimd.collective_compute(
                "AllGather", mybir.AluOpType.bypass,
                replica_groups=[list(range(NC))],
                ins=[dum_in[:, :]], outs=[dum_out[:, :]])

            _phase1_qkv(nc, tc, xT, wq, wk, wv,
                        qt_sb, kt_sb, vt_sb, vn_sb, cos_sb, sin_sb, id_sb)

            with (
                tc.tile_pool(name="attn", bufs=6) as ap,
                tc.tile_pool(name="eacc", bufs=2) as ep,
                tc.tile_pool(name="attops", bufs=3, space="PSUM") as sps,
                tc.tile_pool(name="attacc", bufs=1, space="PSUM") as acc_ps,
                tc.tile_pool(name="attout", bufs=3) as aop,
                tc.tile_pool(name="ostrip", bufs=4) as osp,
                tc.tile_pool(name="ops", bufs=1, space="PSUM") as ops,
                tc.tile_pool(name="oout", bufs=2) as oop,
            ):
                def ph2(qc, fillers=()):
                    _phase2_chunk(nc, tc, ap, ep, sps, acc_ps, aop,
                                  qt_sb, kt_sb, vn_sb, tri_sb, ones_sb,
                                  onesr_sb, att_loc[qc], qc, inv_sqrt_d,
                                  fillers)

                def gather(qc):
                    nc.gpsimd.collective_compute(
                        "AllGather",
                        mybir.AluOpType.bypass,
                        replica_groups=[list(range(NC))],
                        ins=[att_loc[qc][:, :]],
                        outs=[att_full[qc][:, :]],
                    )

                strips = {}

                def load_strips(qc, spread=False):
                    """4 per-t-tile strips of chunk qc. Normally on the sync
                    queue, called at the start of ph2(qc+2) by which time
                    AG(qc) completed so the queue never blocks. With spread
                    (endgame, other queues idle) one DMA per engine so all
                    four strips transfer in parallel."""
                    att_r = att_full[qc].rearrange("(a p) t -> p a t", p=128)
                    engs = ([nc.sync, nc.gpsimd, nc.scalar, nc.sync]
                            if spread else [nc.sync] * 4)
                    sts = []
                    for tt in range(4):
                        st = osp.tile([128, KT * 128], BF16, tag="strip")
                        engs[tt].dma_start(
                            st[:, :].rearrange("p (a f) -> p a f", a=KT),
                            att_r[:, :, tt * 128:(tt + 1) * 128])
                        sts.append(st)
                    strips[qc] = sts

                def ph3_fillers(qc):
                    """8 closures, each emitting half an o_proj psum tile."""
                    fs = []
                    for tt in range(4):
                        holder = {}

                        def first(tt=tt, holder=holder):
                            holder["ps"] = _ph3_half(
                                nc, ops, strips[qc][tt], wo_sb, 0, None)

                        def second(tt=tt, holder=holder, qc=qc):
                            o_ps = _ph3_half(
                                nc, ops, strips[qc][tt], wo_sb, 1, holder["ps"])
                            ot = oop.tile([128, 512], F32, tag="ot")
                            nc.scalar.copy(ot[:, :], o_ps[:, :])
                            row = (qc * 4 + tt) * 128
                            nc.sync.dma_start(out[row:row + 128, :], ot[:, :])
                        fs += [first, second]
                    return fs

                ph2(0)
                gather(0)
                ph2(1)
                gather(1)
                load_strips(0)
                ph2(2, fillers=ph3_fillers(0))
                gather(2)
                load_strips(1)
                ph2(3, fillers=ph3_fillers(1))
                gather(3)
                load_strips(2, spread=True)
                for f in ph3_fillers(2):
                    f()
                load_strips(3, spread=True)
                for f in ph3_fillers(3):
                    f()

    nc.compile()
    return nc


def _ph3_half(nc, ops, strip, wo_sb, half, o_ps):
    """Emit 16 of the 32 accumulating o_proj matmuls for one t-tile strip."""
    if o_ps is None:
        o_ps = ops.tile([128, 512], F32, tag="ops")
    for k2 in range(half * 16, half * 16 + 16):
        nc.tensor.matmul(
            o_ps[:, :],
            strip[:, k2 * 128:(k2 + 1) * 128],
            wo_sb[:, k2 * DQ:(k2 + 1) * DQ],
            start=(k2 == 0), stop=(k2 == KT - 1),
            skip_group_check=True)
    return o_ps


def _phase1_qkv(nc, tc, xT, wq, wk, wv,
                qt_sb, kt_sb, vt_sb, vn_sb, cos_sb, sin_sb, id_sb):
    with tc.tile_pool(name="wqkv", bufs=1) as wp:
        wq_sb = wp.tile([128, KT * DQ], BF16, tag="wq")
        wk_sb = wp.tile([128, KT * D], BF16, tag="wk")
        wv_sb = wp.tile([128, KT * D], BF16, tag="wv")
        nc.sync.dma_start(
            wq_sb[:, :].rearrange("p (a m) -> p a m", a=KT),
            wq.rearrange("(a p) m -> p a m", p=128))
        nc.sync.dma_start(
            wk_sb[:, :].rearrange("p (a m) -> p a m", a=KT),
            wk.rearrange("(a p) m -> p a m", p=128))
        nc.sync.dma_start(
            wv_sb[:, :].rearrange("p (a m) -> p a m", a=KT),
            wv.rearrange("(a p) m -> p a m", p=128))

        with (
            tc.tile_pool(name="xrhs", bufs=4) as xp,
            tc.tile_pool(name="qkvps", bufs=1, space="PSUM") as qps,
            tc.tile_pool(name="ropem", bufs=5) as rpm,
            tc.tile_pool(name="ropes", bufs=2) as rps,
            tc.tile_pool(name="vtp", bufs=2, space="PSUM") as vps,
        ):
            for tcn in range(TC):
                ts = tcn * 512
                pq = [qps.tile([128, 512], F32, tag=f"pq{h}", name=f"pq{h}")
                      for h in range(HQ)]
                pk = qps.tile([128, 512], F32, tag="pk")
                pv = qps.tile([128, 512], F32, tag="pv")
                for k in range(KT):
                    xt = xp.tile([128, 512], BF16, tag="xt")
                    nc.sync.dma_start(
                        xt[:, :], xT[k * 128:(k + 1) * 128, ts:ts + 512])
                    for h in range(HQ):
                        nc.tensor.matmul(
                            pq[h][:, :],
                            wq_sb[:, k * DQ + h * 128: k * DQ + (h + 1) * 128],
                            xt[:, :],
                            start=(k == 0), stop=(k == KT - 1),
                        )
                    nc.tensor.matmul(
                        pk[:, :], wk_sb[:, k * D:(k + 1) * D], xt[:, :],
                        start=(k == 0), stop=(k == KT - 1))
                    nc.tensor.matmul(
                        pv[:, :], wv_sb[:, k * D:(k + 1) * D], xt[:, :],
                        start=(k == 0), stop=(k == KT - 1))

                # epilogue. Free the 6 PSUM banks ASAP: per target the DVE
                # cos-mult and ACT copy lead; shuffle/sin/add trail.
                nc.scalar.copy(vt_sb[:, ts:ts + 512], pv[:, :])
                qc_ts, qn_ts = [], []
                for h in range(HQ + 1):
                    src = pq[h] if h < HQ else pk
                    qc_t = rpm.tile([128, 512], F32, tag="qcos")
                    nc.vector.tensor_tensor(
                        qc_t[:, :], src[:, :], cos_sb[:, ts:ts + 512],
                        op=mybir.AluOpType.mult)
                    qn_t = rpm.tile([128, 512], F32, tag="qnat")
                    nc.scalar.copy(qn_t[:, :], src[:, :])
                    qc_ts.append(qc_t)
                    qn_ts.append(qn_t)

                # V transpose for this chunk fills the PE boundary bubble
                for t4 in range(4):
                    tb = ts + t4 * 128
                    vp = vps.tile([128, 128], BF16, tag="vtp")
                    nc.tensor.transpose(
                        vp[:, :], vt_sb[:, tb:tb + 128], id_sb[:, :])
                    nc.scalar.copy(vn_sb[:, tb:tb + 128], vp[:, :])

                for h in range(HQ + 1):
                    dst = qt_sb[h] if h < HQ else kt_sb
                    sh_t = rps.tile([128, 512], F32, tag="qshuf")
                    nc.gpsimd.dma_start(sh_t[0:64, :], qn_ts[h][64:128, :])
                    nc.gpsimd.dma_start(sh_t[64:128, :], qn_ts[h][0:64, :])
                    ss_t = rps.tile([128, 512], F32, tag="qsin")
                    eng = nc.vector
                    eng.tensor_tensor(
                        ss_t[:, :], sh_t[:, :], sin_sb[:, ts:ts + 512],
                        op=mybir.AluOpType.mult)
                    eng.tensor_tensor(
                        dst[:, ts:ts + 512], qc_ts[h][:, :], ss_t[:, :],
                        op=mybir.AluOpType.add)


def _phase2_chunk(nc, tc, ap, ep, sps, acc_ps, aop,
                  qt_sb, kt_sb, vn_sb, tri_sb, ones_sb, onesr_sb,
                  att_loc_qc, qc, inv_sqrt_d, fillers):
    """Attention for q-chunk qc (512 queries), all HQ heads together.

    fillers: list of closures (o_proj work of an older chunk) interleaved
    into the kt loop to keep the PE dense while the S->exp->PV chains drain.
    """
    qs = qc * 512
    n_kt = 4 * (qc + 1)
    o_ps = [acc_ps.tile([128, 512], F32, tag=f"opv{h}", name=f"opv{h}")
            for h in range(HQ)]
    e_acc = [ep.tile([128, 512], F32, tag=f"eacc{h}", name=f"eacc{h}")
             for h in range(HQ)]

    def ncols(kt):
        m = kt - 4 * qc
        return 512 - 128 * m if m >= 0 else 512

    def s_exp(kt, h):
        """S matmul + exp (+ tri mask on the leading 128 cols of diagonal
        tiles, DVE) + denominator accumulate on DVE."""
        n = ncols(kt)
        off = 512 - n
        s_ps = sps.tile([128, 512], F32, tag="st")
        nc.tensor.matmul(
            s_ps[:, 0:n],
            kt_sb[:, kt * 128:(kt + 1) * 128],
            qt_sb[h][:, qs + off:qs + 512],
            start=True, stop=True, skip_group_check=True)
        e_t = ap.tile([128, 512], BF16, tag=f"et{h}", name=f"et{h}")
        nc.scalar.activation(
            e_t[:, 0:n], s_ps[:, 0:n],
            mybir.ActivationFunctionType.Exp,
            scale=inv_sqrt_d)
        if kt - 4 * qc >= 0:  # diagonal tile: mask leading 128 cols
            nc.vector.tensor_tensor(
                e_t[:, 0:128], e_t[:, 0:128], tri_sb[:, :],
                op=mybir.AluOpType.mult)
        eng = nc.vector
        if kt == 0:
            eng.tensor_scalar_add(e_acc[h][:, :], e_t[:, :], 0.0)
        else:
            eng.tensor_tensor(
                e_acc[h][:, off:512], e_acc[h][:, off:512], e_t[:, 0:n],
                op=mybir.AluOpType.add)
        return e_t

    def pv(kt, e_ts):
        n = ncols(kt)
        off = 512 - n
        for h in range(HQ):
            nc.tensor.matmul(
                o_ps[h][:, off:512],
                vn_sb[:, kt * 128:(kt + 1) * 128],
                e_ts[h][:, 0:n],
                start=(kt == 0), stop=(kt == n_kt - 1),
                skip_group_check=True)

    fillers = list(fillers)
    nfill = len(fillers)
    fill_done = 0

    prev = None
    for kt in range(n_kt):
        cur = [s_exp(kt, h) for h in range(HQ)]
        if prev is not None:
            pv(kt - 1, prev)
        prev = cur
        # interleave o_proj filler work of an older chunk
        want = nfill * (kt + 1) // n_kt
        while fill_done < want:
            fillers[fill_done]()
            fill_done += 1
    pv(n_kt - 1, prev)

    # denominator: bf16 convert on DVE, then 4 col-tiled bf16 matmuls
    # partition-reduce into one PSUM bank
    den_ps = sps.tile([128, 512], F32, tag="st")
    for h in range(HQ):
        e_bf = ap.tile([128, 512], BF16, tag=f"ebf{h}", name=f"ebf{h}")
        nc.vector.tensor_scalar_add(e_bf[:, :], e_acc[h][:, :], 0.0)
        nc.tensor.matmul(
            den_ps[32 * h:32 * h + 1, :],
            ones_sb[:, :], e_bf[:, :],
            start=True, stop=True,
            skip_group_check=True, tile_position=(0, 32 * h))

    # normalize + store transposed attention chunk.
    # all reciprocals first so the PE rb matmuls run gapless, then the
    # per-head multiply/store chains.
    rb_sbs = []
    for h in range(HQ):
        rc_t = aop.tile([1, 512], F32R, tag="recip")
        with nc.allow_low_precision(reason="f32r is bitwise f32"):
            nc.vector.reciprocal(rc_t[:, :], den_ps[32 * h:32 * h + 1, :])
        rb_ps = sps.tile([128, 512], F32, tag="st")
        nc.tensor.matmul(
            rb_ps[:, :], onesr_sb[:, :], rc_t[:, :],
            start=True, stop=True, skip_group_check=True)
        rb_sb = aop.tile([128, 512], BF16, tag=f"rbb{h}", name=f"rbb{h}")
        nc.scalar.copy(rb_sb[:, :], rb_ps[:, :])
        rb_sbs.append(rb_sb)
    for h in range(HQ):
        at_t = aop.tile([128, 512], BF16, tag="attT")
        nc.vector.tensor_tensor(
            at_t[:, :], o_ps[h][:, :], rb_sbs[h][:, :],
            op=mybir.AluOpType.mult)
        nc.sync.dma_start(
            att_loc_qc[h * 128:(h + 1) * 128, :], at_t[:, :])


def _host_consts():
    # rope tables, transposed + sign-folded
    inv = 1.0 / (ROPE_BASE ** (np.arange(0, D, 2, dtype=np.float32) / D))
    t = np.arange(T, dtype=np.float32)
    f = np.outer(t, inv)
    e = np.concatenate([f, f], axis=-1)
    cos = np.cos(e).astype(np.float32)
    sin = np.sin(e).astype(np.float32)
    sgn = np.where(np.arange(D) < D // 2, -1.0, 1.0).astype(np.float32)
    cosT = np.ascontiguousarray(cos.T)
    sinT = np.ascontiguousarray((sin * sgn).T)
    # causal 0/1 triangle for the leading 128 cols of a diagonal tile
    p = np.arange(128)[:, None]
    fr = np.arange(128)[None, :]
    tri = (fr - p >= 0).astype(NPBF16)
    ones = np.ones((128, 1), NPBF16)
    onesr = np.ones((1, 128), np.float32)
    ident = np.eye(128, dtype=NPBF16)
    return cosT, sinT, tri, ones, onesr, ident


def kernel(x, wq, wk, wv, wo, mask=None, **_ignored):
    x = np.asarray(x, dtype=np.float32)
    wq = np.asarray(wq, dtype=np.float32)
    wk = np.asarray(wk, dtype=np.float32)
    wv = np.asarray(wv, dtype=np.float32)
    wo = np.asarray(wo, dtype=np.float32)
    B = x.shape[0]
    xT = np.ascontiguousarray(x.reshape(T, HID).T).astype(NPBF16)   # [HID, T]
    cosT, sinT, tri, ones, onesr, ident = _host_consts()

    if "nc" not in _BUILD_CACHE:
        _BUILD_CACHE["nc"] = _build_nc()
    nc = _BUILD_CACHE["nc"]

    in_maps = []
    for i in range(NC):
        in_maps.append({
            "xT": xT,
            "wq": np.ascontiguousarray(wq[:, i * DQ:(i + 1) * DQ]).astype(NPBF16),
            "wk": np.ascontiguousarray(wk[:, i * D:(i + 1) * D]).astype(NPBF16),
            "wv": np.ascontiguousarray(wv[:, i * D:(i + 1) * D]).astype(NPBF16),
            "wo": np.ascontiguousarray(wo[:, i * DQ:(i + 1) * DQ]).astype(NPBF16),
            "cosT": cosT, "sinT": sinT, "tri": tri, "ones": ones,
            "onesr": onesr, "ident": ident,
        })

    res = run_bass_kernel_spmd(nc, in_maps, core_ids=list(range(NC)), **RUN_KWARGS)
    _BUILD_CACHE["last_res"] = res
    out = np.concatenate([res.results[i]["out"] for i in range(NC)], axis=1)
    return out.reshape(B, T, HID)


if __name__ == "__main__":
    rng = np.random.default_rng(0)
    s = 1.0 / math.sqrt(HID)
    x = rng.standard_normal((1, T, HID), dtype=np.float32)
    wq_ = rng.standard_normal((HID, H * D), dtype=np.float32) * s
    wk_ = rng.standard_normal((HID, KV * D), dtype=np.float32) * s
    wv_ = rng.standard_normal((HID, KV * D), dtype=np.float32) * s
    wo_ = rng.standard_normal((HID, H * D), dtype=np.float32) * s
    o = kernel(x, wq_, wk_, wv_, wo_, None)
    print("out", o.shape, o.dtype, float(np.abs(o).mean()))
